# revision 17
# baseline (speedup 1.0000x reference)
"""Multi-head attention (B=2, S=2048, H=1024, 16 heads) on 8 TRN2 NeuronCores.

Sharding: tensor-parallel over heads x data-parallel over batch.
core = b * 4 + g handles batch b and head-group g (4 heads, 256 channels).

Device-side dataflow (bf16 operands, fp32 PSUM accumulation):
  - Everything stays in "transposed space" so every matmul contracts over the
    partition dim with no on-device transposes:
      x_t    [H, S]      = hidden[b].T                      (host-transposed)
      qk_T   [512, S]    = (Wqk_g x_t)                      rows: q(4 heads), k(4 heads)
      v      [S, 256]    = x w_v.T  (natural layout; lhsT = x_t chunks)
      st     [128k, q]   = k_T_h^T-contracted scores (transposed scores)
      pt     = exp(st * scale + mask[k])                    (ACT, bias = per-partition mask)
      av     [128, q]    = v_aug^T pt ; rows 0:64 = unnormalized out.T,
                           rows 64:128 = Z[q] replicated (v_aug cols 64:128 == 1)
      attn_T [256, S]    = av[:64] * reciprocal(av[64:128])
      out_t  [H, S]      = Wo_g^T-contracted partial output (transposed)
  - Host sums the 4 group partials per batch, transposes back, and adds the
    exact bias corrections: b_out plus w_out @ b_v (the ones-augmented-V
    identity makes the v-bias a constant channel offset).
"""

import numpy as np

import concourse.tile as tile
from concourse import bacc, mybir
from concourse.bass_utils import run_bass_kernel_spmd

B, S, H = 2, 2048, 1024
NH, HD = 16, 64
NCORES = 8
NGROUP = 4              # head groups = cores per batch
HPG = NH // NGROUP      # 4 heads per group
DG = HPG * HD           # 256 channels per group
P = 128
SCALE = float(HD) ** -0.5

FP32 = mybir.dt.float32
FP32R = mybir.dt.float32r
BF16 = mybir.dt.bfloat16

S_TILES = S // P        # 16 key/token tiles
WCOL = {0: 0, 2: 1, 1: 2, 3: 3}  # qk row-chunk -> wqk_t column chunk
HC = H // P             # 8 contraction chunks over H
QKR = 2 * DG            # 512 q+k rows
QKC = QKR // P          # 4 chunks of qk rows
TQ = 512                # token quarter for qkv streaming
NQT = S // TQ           # 4
QT = 1024               # q tile width in attention / out-proj
NQ = S // QT            # 2

_NC_CACHE = None
LAST_RESULT = None      # BassKernelResults of the most recent run (for test.py)


def _body(tc, x_t, wqk_t, wv_t, wo_t, bqk, mask, out_t):
    nc = tc.nc
    with (
        tc.tile_pool(name="const", bufs=1) as const,
        tc.tile_pool(name="big", bufs=1) as big,
        tc.tile_pool(name="pt_pool", bufs=34) as pt_pool,
        tc.tile_pool(name="rz_pool", bufs=2) as rz_pool,
        tc.tile_pool(name="osb_pool", bufs=3) as osb_pool,
        tc.tile_pool(name="ps", bufs=2, space="PSUM") as ps,
        tc.tile_pool(name="avps", bufs=2, space="PSUM") as avps,
        tc.tile_pool(name="iops", bufs=2, space="PSUM") as iops,
    ):
        # ---------- input DMAs ----------
        # wqk FIRST (first matmuls need it); x chunks split over sync/vector
        # so they land progressively for the hc-major prologue; nothing
        # latency-critical on gpsimd (its engine boot is ~10us).
        # byte-balanced across the three HWDGE engines; x chunks staggered
        # so the hc-major prologue consumes them in arrival order.
        wqk_sb = const.tile([P, HC, QKR], BF16, name="wqk_sb")
        wv_sb = const.tile([P, HC, DG], BF16, name="wv_sb")
        bqk_sb = const.tile([P, QKC], FP32, name="bqk_sb")
        mask_sb = const.tile([P, S_TILES], FP32, name="mask_sb")
        wo_sb = const.tile([P, DG // P, H], BF16, name="wo_sb")
        x_sb = big.tile([P, HC, S], BF16, name="x_sb")
        x_r = x_t.rearrange("(c p) s -> p c s", p=P)

        def xdma(eng, hc):
            eng.dma_start(x_sb[:, hc, :], x_r[:, hc, :])

        # wqk halves: [q0|k0] (all the prologue needs, contiguous) first;
        # [q1|k1] right behind on another engine.
        wqk_r = wqk_t.rearrange("(c p) r -> p c r", p=P)
        nc.sync.dma_start(wqk_sb[:, :, 0:2 * P], wqk_r[:, :, 0:2 * P])
        nc.gpsimd.dma_start(bqk_sb[:], bqk.rearrange("(c p) -> p c", p=P))
        nc.gpsimd.dma_start(mask_sb[:], mask.rearrange("(c p) -> p c", p=P))
        nc.scalar.dma_start(wqk_sb[:, :, 2 * P:4 * P], wqk_r[:, :, 2 * P:4 * P])
        nc.gpsimd.dma_start(wv_sb[:], wv_t.rearrange("(c p) r -> p c r", p=P))
        xdma(nc.sync, 1)
        xdma(nc.scalar, 0)
        xdma(nc.gpsimd, 3)
        xdma(nc.sync, 4)
        xdma(nc.scalar, 2)
        xdma(nc.gpsimd, 6)
        xdma(nc.sync, 7)
        xdma(nc.scalar, 5)
        nc.gpsimd.dma_start(wo_sb[:], wo_t.rearrange("(c p) r -> p c r", p=P))

        qk_sb = big.tile([P, QKC, S], BF16, name="qk_sb")
        # v_aug: per token tile / head: [v (64 cols) | ones (64 cols)]
        v_sb = big.tile([P, S_TILES, HPG, 2 * HD], BF16, name="v_sb")
        attn_sb = big.tile([P, DG // P, S], BF16, name="attn_sb")

        # ones half of v_aug: memset a bf16 staging tile, copy per token tile
        ones_sb = const.tile([P, HPG, HD], BF16, name="ones_sb")
        nc.vector.memset(ones_sb[:], 1.0)
        for tt in range(S_TILES):
            nc.vector.tensor_copy(v_sb[:, tt, :, HD:2 * HD], ones_sb[:])

        # ---------- hc-major prologue ----------
        # Only what the very first score tiles need: q pair0 window 0
        # (qk rc=0 i=0) and k pair0 tiles 0-7 (qk rc=2 i=0,1), accumulated
        # chunk-major so each x chunk is consumed as its DMA lands.  The
        # rest of the projections stream in as metered filler below.
        pro_a = ps.tile([P, QT], FP32, name="st", tag="mm")
        pro_b = ps.tile([P, QT], FP32, name="st", tag="mm")
        pro_c = avps.tile([P, 512], FP32, name="av0", tag="av")
        pro_v0 = iops.tile([P, 512], FP32, name="v_ps", tag="io")
        pro_v1 = iops.tile([P, 512], FP32, name="v_ps", tag="io")
        for hc in range(HC):
            se = (hc == 0, hc == HC - 1)
            nc.tensor.matmul(pro_a[:, 0:512], lhsT=wqk_sb[:, hc, 0:P],
                             rhs=x_sb[:, hc, 0:512], start=se[0], stop=se[1])
            wk0 = wqk_sb[:, hc, P:2 * P]
            for dst, i in ((pro_a[:, 512:1024], 0), (pro_b[:, 0:512], 1),
                           (pro_b[:, 512:1024], 2), (pro_c[:], 3)):
                nc.tensor.matmul(dst, lhsT=wk0,
                                 rhs=x_sb[:, hc, i * 512:(i + 1) * 512],
                                 start=se[0], stop=se[1])
            for vps, tt in ((pro_v0, 0), (pro_v1, 1)):
                nc.tensor.matmul(
                    vps[:, 0:DG],
                    lhsT=x_sb[:, hc, tt * P:(tt + 1) * P],
                    rhs=wv_sb[:, hc, :], start=se[0], stop=se[1])
        nc.vector.tensor_scalar_add(qk_sb[:, 0, 0:512], pro_a[:, 0:512],
                                    bqk_sb[:, 0:1])
        nc.vector.tensor_scalar_add(qk_sb[:, 2, 0:512], pro_a[:, 512:1024],
                                    bqk_sb[:, 2:3])
        nc.vector.tensor_scalar_add(qk_sb[:, 2, 512:1024], pro_b[:, 0:512],
                                    bqk_sb[:, 2:3])
        nc.vector.tensor_scalar_add(qk_sb[:, 2, 1024:1536], pro_b[:, 512:1024],
                                    bqk_sb[:, 2:3])
        nc.vector.tensor_scalar_add(qk_sb[:, 2, 1536:2048], pro_c[:],
                                    bqk_sb[:, 2:3])
        for vps, tt in ((pro_v0, 0), (pro_v1, 1)):
            nc.vector.tensor_copy(
                v_sb[:, tt, :, 0:HD],
                vps[:, 0:DG].rearrange("p (h d) -> p h d", d=HD))

        # ---------- static micro-scheduler ----------
        # One flat PE instruction stream: score(t) every ~997ns (the exp
        # cadence); everything else (remaining qkv projection, AV
        # accumulation, out-proj) is sliced into <=2-matmul pieces and
        # packed into the gaps under an explicit PE-time budget, so the
        # in-order PE FIFO never delays a score (and hence never starves
        # the ACT exp stream).
        MM512 = 0.215   # us, N=512 matmul issue slot (warm)
        MM256 = 0.110
        EXPP = 0.997    # exp cadence
        o_r = out_t.rearrange("(c p) s -> p c s", p=P)
        NT = 8 * S_TILES

        def win(w):
            return (0, w) if w < 4 else (1, w - 4)

        # --- filler piece generators (qk / v groups sliced hc-pair-wise) ---
        filler_q = []   # (cost_us, fn)

        def gen_qk(rc, i):
            st_ = {}

            def mk(h0):
                def f():
                    if "t" not in st_:
                        st_["t"] = iops.tile([P, 512], FP32, name="qk_ps",
                                             tag="io")
                    for hc in (h0, h0 + 1):
                        nc.tensor.matmul(
                            st_["t"][:],
                            lhsT=wqk_sb[:, hc,
                                        WCOL[rc] * P:(WCOL[rc] + 1) * P],
                            rhs=x_sb[:, hc, i * 512:(i + 1) * 512],
                            start=(hc == 0), stop=(hc == HC - 1))
                return f
            for h0 in range(0, HC, 2):
                filler_q.append((2 * MM512, mk(h0)))

            def fin():
                nc.vector.tensor_scalar_add(
                    qk_sb[:, rc, i * 512:(i + 1) * 512], st_["t"][:],
                    bqk_sb[:, rc:rc + 1])
                done_groups.add(("qk", rc, i))
            filler_q.append((0.0, fin))

        v_ready = {}    # token tile -> modeled pe time its SBUF copy lands
        done_groups = {("qk", 0, 0), ("qk", 2, 0), ("qk", 2, 1),
                       ("qk", 2, 2), ("qk", 2, 3)}
        v_ready[0] = v_ready[1] = 0.0

        def gen_v(tp):
            st_ = {}

            def mk(half, h0):
                def f():
                    if "t" not in st_:
                        st_["t"] = iops.tile([P, 512], FP32, name="v_ps",
                                             tag="io")
                    tt = 2 * tp + half
                    for hc in (h0, h0 + 1):
                        nc.tensor.matmul(
                            st_["t"][:, half * DG:(half + 1) * DG],
                            lhsT=x_sb[:, hc, tt * P:(tt + 1) * P],
                            rhs=wv_sb[:, hc, :],
                            start=(hc == 0), stop=(hc == HC - 1))
                return f
            for half in range(2):
                for h0 in range(0, HC, 2):
                    filler_q.append((2 * MM256, mk(half, h0)))

            def fin():
                nc.vector.tensor_copy(
                    v_sb[:, 2 * tp:2 * tp + 2, :, 0:HD],
                    st_["t"][:].rearrange("p (t h d) -> p t h d", t=2, d=HD))
                v_ready[2 * tp] = v_ready[2 * tp + 1] = pe[0] + 0.9
            filler_q.append((0.0, fin))

        # filler order: k tiles 8-15 first (scores slots 8-15 need them),
        # v in token order (AV consumption), q chunks before their windows,
        # pair-1 k before slot 64.
        gen_v(1)
        gen_v(2)
        gen_qk(0, 1)
        gen_v(3)
        gen_qk(3, 0)
        gen_v(4)
        gen_qk(0, 2)
        gen_v(5)
        gen_qk(3, 1)
        gen_v(6)
        gen_qk(3, 2)
        gen_v(7)
        gen_qk(0, 3)
        gen_qk(3, 3)
        for j in range(4):
            gen_qk(1, j)

        # --- scheduler state ---
        pe = [0.0]          # modeled PE-busy time since stream start
        E = [-EXPP]         # E[t+1] = modeled end of exp(t)
        cur_t = [0]         # current slot (wall-clock proxy for gates)
        avs = {}
        pts = {}
        av_q = []           # pending AV units: (t, kt, w)
        op_q = []           # pending out-proj: (ready_pe, ready_slot, q5, j)
        fin_pe = {}
        fin_slot = {}

        def emit_score_exp(t):
            w, kt = divmod(t, S_TILES)
            qc, q5 = win(w)
            qlo = q5 * 512
            st = ps.tile([P, QT], FP32, name="st", tag="mm")
            for half in range(2):
                off = half * HD
                nc.tensor.matmul(
                    st[:, half * 512:(half + 1) * 512],
                    lhsT=qk_sb[off:off + HD, 2 + qc, kt * P:(kt + 1) * P],
                    rhs=qk_sb[off:off + HD, qc, qlo:qlo + 512],
                    start=True, stop=True)
            pe[0] += MM512
            pt = pt_pool.tile([P, QT], BF16, name="pt", tag="pt")
            nc.scalar.activation(
                pt[:], st[:], mybir.ActivationFunctionType.Exp,
                bias=mask_sb[:, kt:kt + 1], scale=SCALE)
            pts[t] = pt
            E.append(max(E[-1] + EXPP, pe[0] + 0.45))

        def av_ready(unit):
            t, kt, w = unit
            if pe[0] + 0.1 < E[t + 1]:          # exp(t) must have completed
                return False
            if v_ready.get(kt, 1e9) > pe[0]:    # v tile must be in SBUF
                return False
            if kt == 0 and w > 0:               # av slots from window w-1:
                if w - 1 not in fin_pe:         # DVE chain needs ~3.4us of
                    return False                # either PE-busy or wall time
                if (pe[0] < fin_pe[w - 1] + 3.4
                        and cur_t[0] < fin_slot[w - 1] + 4):
                    return False
            return True

        def emit_av(unit):
            t, kt, w = unit
            qc, q5 = win(w)
            if kt == 0:
                avs[w] = (avps.tile([P, 512], FP32, name="av0", tag="av"),
                          avps.tile([P, 512], FP32, name="av1", tag="av"))
            pt = pts.pop(t)
            for half, av in ((0, avs[w][0]), (1, avs[w][1])):
                nc.tensor.matmul(
                    av[:], lhsT=v_sb[:, kt, 2 * qc + half, :],
                    rhs=pt[:, half * 512:(half + 1) * 512],
                    start=(kt == 0), stop=(kt == S_TILES - 1))
            pe[0] += 2 * MM512
            if kt == S_TILES - 1:
                emit_finalize(w, tail=(w == 7))

        def emit_finalize(w, tail=False):
            qc, q5 = win(w)
            qlo = q5 * 512
            for half, av in ((0, avs[w][0]), (1, avs[w][1])):
                off = half * HD
                zc = rz_pool.tile([HD, 512], FP32, name="zc", tag="zc")
                if tail:
                    nc.scalar.copy(zc[:], av[HD:2 * HD, :])
                else:
                    nc.vector.tensor_copy(zc[:], av[HD:2 * HD, :])
                rz = rz_pool.tile([HD, 512], FP32, name="rz", tag="rz")
                nc.vector.reciprocal_approx_fast(rz[:], zc[:])
                nc.vector.tensor_mul(
                    attn_sb[off:off + HD, qc, qlo:qlo + 512],
                    av[0:HD, :], rz[:])
            del avs[w]
            fin_pe[w] = pe[0]
            fin_slot[w] = cur_t[0]
            if w >= 4:
                q5o = w - 4
                for j in range(H // P):
                    op_q.append((pe[0] + 3.0, cur_t[0] + 4, q5o, j))

        def op_ready(unit):
            return pe[0] >= unit[0] or cur_t[0] >= unit[1]

        def emit_op(unit, tail=False):
            _, _, q5, j = unit
            qlo = q5 * 512
            o_ps = iops.tile([P, 512], FP32, name="o_ps", tag="io")
            for kc in range(DG // P):
                nc.tensor.matmul(
                    o_ps[:], lhsT=wo_sb[:, kc, j * P:(j + 1) * P],
                    rhs=attn_sb[:, kc, qlo:qlo + 512],
                    start=(kc == 0), stop=(kc == DG // P - 1))
            pe[0] += 2 * MM512
            o_sb = osb_pool.tile([P, 512], BF16, name="o_sb", tag="osb")
            if tail:
                nc.scalar.copy(o_sb[:], o_ps[:])
            else:
                nc.vector.tensor_copy(o_sb[:], o_ps[:])
            nc.sync.dma_start(o_r[:, j, qlo:qlo + 512], o_sb[:])

        def fill_until(deadline):
            # round-robin under the exp-cadence budget: AV first (pt-slot
            # recycling gates the exp stream), filler second, out-proj when
            # its window is done.  Then a small unconditional filler quota
            # so projection prerequisites never pile into a forced burst.
            spent_f = 0.0
            while (av_q and av_ready(av_q[0])
                    and pe[0] + 2 * MM512 <= deadline):
                emit_av(av_q.pop(0))
            while filler_q and pe[0] + filler_q[0][0] <= deadline:
                cost, fn = filler_q.pop(0)
                fn(); pe[0] += cost; spent_f += cost
            while (op_q and op_ready(op_q[0])
                    and pe[0] + 2 * MM512 <= deadline):
                emit_op(op_q.pop(0))
            while filler_q and spent_f < 0.45:
                cost, fn = filler_q.pop(0)
                fn(); pe[0] += cost; spent_f += cost

        def need_group(key):
            # force-drain filler (in order) until `key`'s finishing piece
            # has been emitted — scores must never precede their qk writes
            # in emission order (Tile deps follow emission order).
            while key not in done_groups:
                cost, fn = filler_q.pop(0)
                fn(); pe[0] += cost

        for t in range(NT):
            cur_t[0] = t
            w, kt = divmod(t, S_TILES)
            qc, q5 = win(w)
            need_group(("qk", qc, q5))
            need_group(("qk", 2 + qc, kt // 4))
            emit_score_exp(t)
            av_q.append((t, kt, w))
            fill_until(E[-1] - 0.32)

        # drain remaining AV / out-proj work
        while av_q:
            cur_t[0] += 1
            if av_ready(av_q[0]):
                emit_av(av_q.pop(0))
            elif filler_q:
                cost, fn = filler_q.pop(0)
                fn(); pe[0] += cost
            else:
                pe[0] = max(pe[0] + 0.2, E[av_q[0][0] + 1])
        while filler_q:
            cost, fn = filler_q.pop(0)
            fn(); pe[0] += cost
        for unit in op_q:
            emit_op(unit, tail=(unit[2] == 3))
        op_q.clear()


def _build():
    nc = bacc.Bacc(
        "TRN2",
        target_bir_lowering=False,
        debug=False,
        enable_asserts=True,
        num_devices=NCORES,
    )
    x_t = nc.dram_tensor("x_t", [H, S], BF16, kind="ExternalInput").ap()
    wqk_t = nc.dram_tensor("wqk_t", [H, QKR], BF16, kind="ExternalInput").ap()
    wv_t = nc.dram_tensor("wv_t", [H, DG], BF16, kind="ExternalInput").ap()
    wo_t = nc.dram_tensor("wo_t", [DG, H], BF16, kind="ExternalInput").ap()
    bqk = nc.dram_tensor("bqk", [QKR], FP32, kind="ExternalInput").ap()
    mask = nc.dram_tensor("mask", [S], FP32, kind="ExternalInput").ap()
    out_t = nc.dram_tensor("out_t", [H, S], BF16, kind="ExternalOutput").ap()

    with tile.TileContext(nc) as tc:
        _body(tc, x_t, wqk_t, wv_t, wo_t, bqk, mask, out_t)
    nc.compile()
    return nc


def _get_nc():
    global _NC_CACHE
    if _NC_CACHE is None:
        _NC_CACHE = _build()
    return _NC_CACHE


def make_in_maps(hidden_states, attention_mask, w_qkv, b_qkv, w_out):
    import ml_dtypes

    bf16 = ml_dtypes.bfloat16
    in_maps = []
    for core in range(NCORES):
        b, g = divmod(core, NGROUP)
        wq = w_qkv[0 * H + g * DG:0 * H + (g + 1) * DG]
        wk = w_qkv[1 * H + g * DG:1 * H + (g + 1) * DG]
        wv = w_qkv[2 * H + g * DG:2 * H + (g + 1) * DG]
        in_maps.append({
            "x_t": np.ascontiguousarray(hidden_states[b].T).astype(bf16),
            # column order [q0|k0|q1|k1] (128-row chunks interleaved) so
            # the kernel can DMA the first-needed half contiguously
            "wqk_t": np.ascontiguousarray(
                np.concatenate([wq[:128], wk[:128], wq[128:], wk[128:]],
                               0).T).astype(bf16),
            "wv_t": np.ascontiguousarray(wv.T).astype(bf16),
            "wo_t": np.ascontiguousarray(
                w_out[:, g * DG:(g + 1) * DG].T).astype(bf16),
            "bqk": np.ascontiguousarray(
                np.concatenate([b_qkv[g * DG:(g + 1) * DG],
                                b_qkv[H + g * DG:H + (g + 1) * DG]])),
            "mask": np.ascontiguousarray(attention_mask[b]),
        })
    return in_maps


def kernel(hidden_states, attention_mask, w_qkv, b_qkv, w_out, b_out):
    global LAST_RESULT
    hidden_states = np.asarray(hidden_states, dtype=np.float32)
    attention_mask = np.asarray(attention_mask, dtype=np.float32)
    w_qkv = np.asarray(w_qkv, dtype=np.float32)
    b_qkv = np.asarray(b_qkv, dtype=np.float32)
    w_out = np.asarray(w_out, dtype=np.float32)
    b_out = np.asarray(b_out, dtype=np.float32)

    nc = _get_nc()
    in_maps = make_in_maps(hidden_states, attention_mask, w_qkv, b_qkv, w_out)

    import os
    trace = bool(int(os.environ.get("KERNEL_TRACE", "0")))
    res = run_bass_kernel_spmd(
        nc, in_maps, core_ids=list(range(NCORES)), trace=trace,
    )
    LAST_RESULT = res

    out = np.zeros((B, S, H), np.float32)
    vbias = w_out @ b_qkv[2 * H:]          # exact v-bias correction
    for b in range(B):
        acc = res.results[b * NGROUP + 0]["out_t"].astype(np.float32)
        for g in range(1, NGROUP):
            acc = acc + res.results[b * NGROUP + g]["out_t"].astype(np.float32)
        out[b] = acc.T + b_out + vbias
    return out



# revision 18
# speedup vs baseline: 1.0069x; 1.0069x over previous
"""Multi-head attention (B=2, S=2048, H=1024, 16 heads) on 8 TRN2 NeuronCores.

Sharding: tensor-parallel over heads x data-parallel over batch.
core = b * 4 + g handles batch b and head-group g (4 heads, 256 channels).

Device-side dataflow (bf16 operands, fp32 PSUM accumulation):
  - Everything stays in "transposed space" so every matmul contracts over the
    partition dim with no on-device transposes:
      x_t    [H, S]      = hidden[b].T                      (host-transposed)
      qk_T   [512, S]    = (Wqk_g x_t)                      rows: q(4 heads), k(4 heads)
      v      [S, 256]    = x w_v.T  (natural layout; lhsT = x_t chunks)
      st     [128k, q]   = k_T_h^T-contracted scores (transposed scores)
      pt     = exp(st * scale + mask[k])                    (ACT, bias = per-partition mask)
      av     [128, q]    = v_aug^T pt ; rows 0:64 = unnormalized out.T,
                           rows 64:128 = Z[q] replicated (v_aug cols 64:128 == 1)
      attn_T [256, S]    = av[:64] * reciprocal(av[64:128])
      out_t  [H, S]      = Wo_g^T-contracted partial output (transposed)
  - Host sums the 4 group partials per batch, transposes back, and adds the
    exact bias corrections: b_out plus w_out @ b_v (the ones-augmented-V
    identity makes the v-bias a constant channel offset).
"""

import numpy as np

import concourse.tile as tile
from concourse import bacc, mybir
from concourse.bass_utils import run_bass_kernel_spmd

B, S, H = 2, 2048, 1024
NH, HD = 16, 64
NCORES = 8
NGROUP = 4              # head groups = cores per batch
HPG = NH // NGROUP      # 4 heads per group
DG = HPG * HD           # 256 channels per group
P = 128
SCALE = float(HD) ** -0.5

FP32 = mybir.dt.float32
FP32R = mybir.dt.float32r
BF16 = mybir.dt.bfloat16

S_TILES = S // P        # 16 key/token tiles
WCOL = {0: 0, 2: 1, 1: 2, 3: 3}  # qk row-chunk -> wqk_t column chunk
HC = H // P             # 8 contraction chunks over H
QKR = 2 * DG            # 512 q+k rows
QKC = QKR // P          # 4 chunks of qk rows
TQ = 512                # token quarter for qkv streaming
NQT = S // TQ           # 4
QT = 1024               # q tile width in attention / out-proj
NQ = S // QT            # 2

_NC_CACHE = None
LAST_RESULT = None      # BassKernelResults of the most recent run (for test.py)


def _body(tc, x_t, wqk_t, wv_t, wo_t, bqk, mask, out_t):
    nc = tc.nc
    with (
        tc.tile_pool(name="const", bufs=1) as const,
        tc.tile_pool(name="big", bufs=1) as big,
        tc.tile_pool(name="pt_pool", bufs=34) as pt_pool,
        tc.tile_pool(name="rz_pool", bufs=2) as rz_pool,
        tc.tile_pool(name="osb_pool", bufs=3) as osb_pool,
        tc.tile_pool(name="ps", bufs=2, space="PSUM") as ps,
        tc.tile_pool(name="avps", bufs=2, space="PSUM") as avps,
        tc.tile_pool(name="iops", bufs=2, space="PSUM") as iops,
    ):
        # ---------- input DMAs ----------
        # wqk FIRST (first matmuls need it); x chunks split over sync/vector
        # so they land progressively for the hc-major prologue; nothing
        # latency-critical on gpsimd (its engine boot is ~10us).
        # byte-balanced across the three HWDGE engines; x chunks staggered
        # so the hc-major prologue consumes them in arrival order.
        wqk_sb = const.tile([P, HC, QKR], BF16, name="wqk_sb")
        wv_sb = const.tile([P, HC, DG], BF16, name="wv_sb")
        bqk_sb = const.tile([P, QKC], FP32, name="bqk_sb")
        mask_sb = const.tile([P, S_TILES], FP32, name="mask_sb")
        wo_sb = const.tile([P, DG // P, H], BF16, name="wo_sb")
        x_sb = big.tile([P, HC, S], BF16, name="x_sb")
        x_r = x_t.rearrange("(c p) s -> p c s", p=P)

        def xdma(eng, hc):
            eng.dma_start(x_sb[:, hc, :], x_r[:, hc, :])

        # wqk halves: [q0|k0] (all the prologue needs, contiguous) first;
        # [q1|k1] right behind on another engine.
        wqk_r = wqk_t.rearrange("(c p) r -> p c r", p=P)
        nc.sync.dma_start(wqk_sb[:, :, 0:2 * P], wqk_r[:, :, 0:2 * P])
        nc.gpsimd.dma_start(bqk_sb[:], bqk.rearrange("(c p) -> p c", p=P))
        nc.gpsimd.dma_start(mask_sb[:], mask.rearrange("(c p) -> p c", p=P))
        nc.scalar.dma_start(wqk_sb[:, :, 2 * P:4 * P], wqk_r[:, :, 2 * P:4 * P])
        nc.gpsimd.dma_start(wv_sb[:], wv_t.rearrange("(c p) r -> p c r", p=P))
        xdma(nc.sync, 1)
        xdma(nc.scalar, 0)
        xdma(nc.gpsimd, 3)
        xdma(nc.sync, 4)
        xdma(nc.scalar, 2)
        xdma(nc.gpsimd, 6)
        xdma(nc.sync, 7)
        xdma(nc.scalar, 5)
        nc.gpsimd.dma_start(wo_sb[:], wo_t.rearrange("(c p) r -> p c r", p=P))

        qk_sb = big.tile([P, QKC, S], BF16, name="qk_sb")
        # v_aug: per token tile / head: [v (64 cols) | ones (64 cols)]
        v_sb = big.tile([P, S_TILES, HPG, 2 * HD], BF16, name="v_sb")
        attn_sb = big.tile([P, DG // P, S], BF16, name="attn_sb")

        # ones half of v_aug: memset a bf16 staging tile, copy per token tile
        ones_sb = const.tile([P, HPG, HD], BF16, name="ones_sb")
        nc.vector.memset(ones_sb[:], 1.0)
        for tt in range(S_TILES):
            nc.vector.tensor_copy(v_sb[:, tt, :, HD:2 * HD], ones_sb[:])

        # ---------- hc-major prologue ----------
        # Only what the very first score tiles need: q pair0 window 0
        # (qk rc=0 i=0) and k pair0 tiles 0-7 (qk rc=2 i=0,1), accumulated
        # chunk-major so each x chunk is consumed as its DMA lands.  The
        # rest of the projections stream in as metered filler below.
        pro_a = ps.tile([P, QT], FP32, name="st", tag="mm")
        pro_b = ps.tile([P, QT], FP32, name="st", tag="mm")
        pro_c = avps.tile([P, 512], FP32, name="av0", tag="av")
        pro_v0 = iops.tile([P, 512], FP32, name="v_ps", tag="io")
        pro_v1 = iops.tile([P, 512], FP32, name="v_ps", tag="io")
        for hc in range(HC):
            se = (hc == 0, hc == HC - 1)
            nc.tensor.matmul(pro_a[:, 0:512], lhsT=wqk_sb[:, hc, 0:P],
                             rhs=x_sb[:, hc, 0:512], start=se[0], stop=se[1])
            wk0 = wqk_sb[:, hc, P:2 * P]
            for dst, i in ((pro_a[:, 512:1024], 0), (pro_b[:, 0:512], 1),
                           (pro_b[:, 512:1024], 2), (pro_c[:], 3)):
                nc.tensor.matmul(dst, lhsT=wk0,
                                 rhs=x_sb[:, hc, i * 512:(i + 1) * 512],
                                 start=se[0], stop=se[1])
            for vps, tt in ((pro_v0, 0), (pro_v1, 1)):
                nc.tensor.matmul(
                    vps[:, 0:DG],
                    lhsT=x_sb[:, hc, tt * P:(tt + 1) * P],
                    rhs=wv_sb[:, hc, :], start=se[0], stop=se[1])
        nc.vector.tensor_scalar_add(qk_sb[:, 0, 0:512], pro_a[:, 0:512],
                                    bqk_sb[:, 0:1])
        nc.vector.tensor_scalar_add(qk_sb[:, 2, 0:512], pro_a[:, 512:1024],
                                    bqk_sb[:, 2:3])
        nc.vector.tensor_scalar_add(qk_sb[:, 2, 512:1024], pro_b[:, 0:512],
                                    bqk_sb[:, 2:3])
        nc.vector.tensor_scalar_add(qk_sb[:, 2, 1024:1536], pro_b[:, 512:1024],
                                    bqk_sb[:, 2:3])
        nc.vector.tensor_scalar_add(qk_sb[:, 2, 1536:2048], pro_c[:],
                                    bqk_sb[:, 2:3])
        for vps, tt in ((pro_v0, 0), (pro_v1, 1)):
            nc.vector.tensor_copy(
                v_sb[:, tt, :, 0:HD],
                vps[:, 0:DG].rearrange("p (h d) -> p h d", d=HD))

        # ---------- static micro-scheduler ----------
        # One flat PE instruction stream: score(t) every ~997ns (the exp
        # cadence); everything else (remaining qkv projection, AV
        # accumulation, out-proj) is sliced into <=2-matmul pieces and
        # packed into the gaps under an explicit PE-time budget, so the
        # in-order PE FIFO never delays a score (and hence never starves
        # the ACT exp stream).
        MM512 = 0.215   # us, N=512 matmul issue slot (warm)
        MM256 = 0.110
        EXPP = 0.997    # exp cadence
        o_r = out_t.rearrange("(c p) s -> p c s", p=P)
        NT = 8 * S_TILES

        def win(w):
            return (0, w) if w < 4 else (1, w - 4)

        # --- filler piece generators (qk / v groups sliced hc-pair-wise) ---
        filler_q = []   # (cost_us, fn)

        def gen_qk(rc, i):
            st_ = {}

            def mk(h0):
                def f():
                    if "t" not in st_:
                        st_["t"] = iops.tile([P, 512], FP32, name="qk_ps",
                                             tag="io")
                    for hc in (h0, h0 + 1):
                        nc.tensor.matmul(
                            st_["t"][:],
                            lhsT=wqk_sb[:, hc,
                                        WCOL[rc] * P:(WCOL[rc] + 1) * P],
                            rhs=x_sb[:, hc, i * 512:(i + 1) * 512],
                            start=(hc == 0), stop=(hc == HC - 1))
                return f
            for h0 in range(0, HC, 2):
                filler_q.append((2 * MM512, mk(h0)))

            def fin():
                nc.vector.tensor_scalar_add(
                    qk_sb[:, rc, i * 512:(i + 1) * 512], st_["t"][:],
                    bqk_sb[:, rc:rc + 1])
                done_groups.add(("qk", rc, i))
            filler_q.append((0.0, fin))

        v_ready = {}    # token tile -> modeled pe time its SBUF copy lands
        done_groups = {("qk", 0, 0), ("qk", 2, 0), ("qk", 2, 1),
                       ("qk", 2, 2), ("qk", 2, 3)}
        v_ready[0] = v_ready[1] = 0.0

        def gen_v(tp):
            st_ = {}

            def mk(half, h0):
                def f():
                    if "t" not in st_:
                        st_["t"] = iops.tile([P, 512], FP32, name="v_ps",
                                             tag="io")
                    tt = 2 * tp + half
                    for hc in (h0, h0 + 1):
                        nc.tensor.matmul(
                            st_["t"][:, half * DG:(half + 1) * DG],
                            lhsT=x_sb[:, hc, tt * P:(tt + 1) * P],
                            rhs=wv_sb[:, hc, :],
                            start=(hc == 0), stop=(hc == HC - 1))
                return f
            for half in range(2):
                for h0 in range(0, HC, 2):
                    filler_q.append((2 * MM256, mk(half, h0)))

            def fin():
                nc.vector.tensor_copy(
                    v_sb[:, 2 * tp:2 * tp + 2, :, 0:HD],
                    st_["t"][:].rearrange("p (t h d) -> p t h d", t=2, d=HD))
                v_ready[2 * tp] = v_ready[2 * tp + 1] = pe[0] + 0.9
            filler_q.append((0.0, fin))

        # filler order: k tiles 8-15 first (scores slots 8-15 need them),
        # v in token order (AV consumption), q chunks before their windows,
        # pair-1 k before slot 64.
        gen_v(1)
        gen_v(2)
        gen_qk(0, 1)
        gen_v(3)
        gen_qk(3, 0)
        gen_v(4)
        gen_qk(0, 2)
        gen_v(5)
        gen_qk(3, 1)
        gen_v(6)
        gen_qk(3, 2)
        gen_v(7)
        gen_qk(0, 3)
        gen_qk(3, 3)
        for j in range(4):
            gen_qk(1, j)

        # --- scheduler state ---
        pe = [0.0]          # modeled PE-busy time since stream start
        E = [-EXPP]         # E[t+1] = modeled end of exp(t)
        cur_t = [0]         # current slot (wall-clock proxy for gates)
        avs = {}
        pts = {}
        av_q = []           # pending AV units: (t, kt, w)
        op_q = []           # pending out-proj: (ready_pe, ready_slot, q5, j)
        fin_pe = {}
        fin_slot = {}

        def emit_score_exp(t):
            w, kt = divmod(t, S_TILES)
            qc, q5 = win(w)
            qlo = q5 * 512
            st = ps.tile([P, QT], FP32, name="st", tag="mm")
            for half in range(2):
                off = half * HD
                nc.tensor.matmul(
                    st[:, half * 512:(half + 1) * 512],
                    lhsT=qk_sb[off:off + HD, 2 + qc, kt * P:(kt + 1) * P],
                    rhs=qk_sb[off:off + HD, qc, qlo:qlo + 512],
                    start=True, stop=True)
            pe[0] += MM512
            pt = pt_pool.tile([P, QT], BF16, name="pt", tag="pt")
            nc.scalar.activation(
                pt[:], st[:], mybir.ActivationFunctionType.Exp,
                bias=mask_sb[:, kt:kt + 1], scale=SCALE)
            pts[t] = pt
            E.append(max(E[-1] + EXPP, pe[0] + 0.45))

        def av_ready(unit):
            t, kt, w = unit
            if pe[0] + 0.1 < E[t + 1]:          # exp(t) must have completed
                return False
            if v_ready.get(kt, 1e9) > pe[0]:    # v tile must be in SBUF
                return False
            if kt == 0 and w > 0:               # av slots: window w-1's
                f = fin_pe.get(w - 1)           # DVE chain must have run
                if f is None or pe[0] < f + 3.4:
                    return False
            return True

        def emit_av(unit):
            t, kt, w = unit
            qc, q5 = win(w)
            if kt == 0:
                avs[w] = (avps.tile([P, 512], FP32, name="av0", tag="av"),
                          avps.tile([P, 512], FP32, name="av1", tag="av"))
            pt = pts.pop(t)
            for half, av in ((0, avs[w][0]), (1, avs[w][1])):
                nc.tensor.matmul(
                    av[:], lhsT=v_sb[:, kt, 2 * qc + half, :],
                    rhs=pt[:, half * 512:(half + 1) * 512],
                    start=(kt == 0), stop=(kt == S_TILES - 1))
            pe[0] += 2 * MM512
            if kt == S_TILES - 1:
                emit_finalize(w, tail=(w == 7))

        def emit_finalize(w, tail=False):
            qc, q5 = win(w)
            qlo = q5 * 512
            for half, av in ((0, avs[w][0]), (1, avs[w][1])):
                off = half * HD
                zc = rz_pool.tile([HD, 512], FP32, name="zc", tag="zc")
                if tail:
                    nc.scalar.copy(zc[:], av[HD:2 * HD, :])
                else:
                    nc.vector.tensor_copy(zc[:], av[HD:2 * HD, :])
                rz = rz_pool.tile([HD, 512], FP32, name="rz", tag="rz")
                nc.vector.reciprocal_approx_fast(rz[:], zc[:])
                nc.vector.tensor_mul(
                    attn_sb[off:off + HD, qc, qlo:qlo + 512],
                    av[0:HD, :], rz[:])
            del avs[w]
            fin_pe[w] = pe[0]
            fin_slot[w] = cur_t[0]
            if w >= 4:
                q5o = w - 4
                for j in range(H // P):
                    op_q.append((pe[0] + 3.0, cur_t[0] + 4, q5o, j))

        def op_ready(unit):
            return pe[0] >= unit[0]

        def emit_op(unit, tail=False):
            _, _, q5, j = unit
            qlo = q5 * 512
            o_ps = iops.tile([P, 512], FP32, name="o_ps", tag="io")
            for kc in range(DG // P):
                nc.tensor.matmul(
                    o_ps[:], lhsT=wo_sb[:, kc, j * P:(j + 1) * P],
                    rhs=attn_sb[:, kc, qlo:qlo + 512],
                    start=(kc == 0), stop=(kc == DG // P - 1))
            pe[0] += 2 * MM512
            o_sb = osb_pool.tile([P, 512], BF16, name="o_sb", tag="osb")
            if tail:
                nc.scalar.copy(o_sb[:], o_ps[:])
            else:
                nc.vector.tensor_copy(o_sb[:], o_ps[:])
            nc.sync.dma_start(o_r[:, j, qlo:qlo + 512], o_sb[:])

        def fill_until(deadline):
            # round-robin under the exp-cadence budget: AV first (pt-slot
            # recycling gates the exp stream), filler second, out-proj when
            # its window is done.  Then a small unconditional filler quota
            # so projection prerequisites never pile into a forced burst.
            spent_f = 0.0
            while (av_q and av_ready(av_q[0])
                    and pe[0] + 2 * MM512 <= deadline):
                emit_av(av_q.pop(0))
            while filler_q and pe[0] + filler_q[0][0] <= deadline:
                cost, fn = filler_q.pop(0)
                fn(); pe[0] += cost; spent_f += cost
            while (op_q and op_ready(op_q[0])
                    and pe[0] + 2 * MM512 <= deadline):
                emit_op(op_q.pop(0))
            while filler_q and spent_f < 0.45:
                cost, fn = filler_q.pop(0)
                fn(); pe[0] += cost; spent_f += cost
            # wall-time floor: when the PE idles (everything gated), the
            # stream still advances -- later-emitted work executes no
            # earlier than the exp frontier, so gates must see that time.
            pe[0] = max(pe[0], E[-1] - 1.2)

        def need_group(key):
            # force-drain filler (in order) until `key`'s finishing piece
            # has been emitted — scores must never precede their qk writes
            # in emission order (Tile deps follow emission order).
            while key not in done_groups:
                cost, fn = filler_q.pop(0)
                fn(); pe[0] += cost

        for t in range(NT):
            cur_t[0] = t
            w, kt = divmod(t, S_TILES)
            qc, q5 = win(w)
            need_group(("qk", qc, q5))
            need_group(("qk", 2 + qc, kt // 4))
            emit_score_exp(t)
            av_q.append((t, kt, w))
            fill_until(E[-1] - 0.32)

        # drain remaining AV / out-proj work
        while av_q:
            cur_t[0] += 1
            if av_ready(av_q[0]):
                emit_av(av_q.pop(0))
            elif filler_q:
                cost, fn = filler_q.pop(0)
                fn(); pe[0] += cost
            else:
                pe[0] = max(pe[0] + 0.2, E[av_q[0][0] + 1])
        while filler_q:
            cost, fn = filler_q.pop(0)
            fn(); pe[0] += cost
        for unit in op_q:
            emit_op(unit, tail=(unit[2] == 3))
        op_q.clear()


def _build():
    nc = bacc.Bacc(
        "TRN2",
        target_bir_lowering=False,
        debug=False,
        enable_asserts=True,
        num_devices=NCORES,
    )
    x_t = nc.dram_tensor("x_t", [H, S], BF16, kind="ExternalInput").ap()
    wqk_t = nc.dram_tensor("wqk_t", [H, QKR], BF16, kind="ExternalInput").ap()
    wv_t = nc.dram_tensor("wv_t", [H, DG], BF16, kind="ExternalInput").ap()
    wo_t = nc.dram_tensor("wo_t", [DG, H], BF16, kind="ExternalInput").ap()
    bqk = nc.dram_tensor("bqk", [QKR], FP32, kind="ExternalInput").ap()
    mask = nc.dram_tensor("mask", [S], FP32, kind="ExternalInput").ap()
    out_t = nc.dram_tensor("out_t", [H, S], BF16, kind="ExternalOutput").ap()

    with tile.TileContext(nc) as tc:
        _body(tc, x_t, wqk_t, wv_t, wo_t, bqk, mask, out_t)
    nc.compile()
    return nc


def _get_nc():
    global _NC_CACHE
    if _NC_CACHE is None:
        _NC_CACHE = _build()
    return _NC_CACHE


def make_in_maps(hidden_states, attention_mask, w_qkv, b_qkv, w_out):
    import ml_dtypes

    bf16 = ml_dtypes.bfloat16
    in_maps = []
    for core in range(NCORES):
        b, g = divmod(core, NGROUP)
        wq = w_qkv[0 * H + g * DG:0 * H + (g + 1) * DG]
        wk = w_qkv[1 * H + g * DG:1 * H + (g + 1) * DG]
        wv = w_qkv[2 * H + g * DG:2 * H + (g + 1) * DG]
        in_maps.append({
            "x_t": np.ascontiguousarray(hidden_states[b].T).astype(bf16),
            # column order [q0|k0|q1|k1] (128-row chunks interleaved) so
            # the kernel can DMA the first-needed half contiguously
            "wqk_t": np.ascontiguousarray(
                np.concatenate([wq[:128], wk[:128], wq[128:], wk[128:]],
                               0).T).astype(bf16),
            "wv_t": np.ascontiguousarray(wv.T).astype(bf16),
            "wo_t": np.ascontiguousarray(
                w_out[:, g * DG:(g + 1) * DG].T).astype(bf16),
            "bqk": np.ascontiguousarray(
                np.concatenate([b_qkv[g * DG:(g + 1) * DG],
                                b_qkv[H + g * DG:H + (g + 1) * DG]])),
            "mask": np.ascontiguousarray(attention_mask[b]),
        })
    return in_maps


def kernel(hidden_states, attention_mask, w_qkv, b_qkv, w_out, b_out):
    global LAST_RESULT
    hidden_states = np.asarray(hidden_states, dtype=np.float32)
    attention_mask = np.asarray(attention_mask, dtype=np.float32)
    w_qkv = np.asarray(w_qkv, dtype=np.float32)
    b_qkv = np.asarray(b_qkv, dtype=np.float32)
    w_out = np.asarray(w_out, dtype=np.float32)
    b_out = np.asarray(b_out, dtype=np.float32)

    nc = _get_nc()
    in_maps = make_in_maps(hidden_states, attention_mask, w_qkv, b_qkv, w_out)

    import os
    trace = bool(int(os.environ.get("KERNEL_TRACE", "0")))
    res = run_bass_kernel_spmd(
        nc, in_maps, core_ids=list(range(NCORES)), trace=trace,
    )
    LAST_RESULT = res

    out = np.zeros((B, S, H), np.float32)
    vbias = w_out @ b_qkv[2 * H:]          # exact v-bias correction
    for b in range(B):
        acc = res.results[b * NGROUP + 0]["out_t"].astype(np.float32)
        for g in range(1, NGROUP):
            acc = acc + res.results[b * NGROUP + g]["out_t"].astype(np.float32)
        out[b] = acc.T + b_out + vbias
    return out



# revision 19
# speedup vs baseline: 1.2624x; 1.2538x over previous
"""Multi-head attention (B=2, S=2048, H=1024, 16 heads) on 8 TRN2 NeuronCores.

Sharding: tensor-parallel over heads x data-parallel over batch.
core = b * 4 + g handles batch b and head-group g (4 heads, 256 channels).

Device-side dataflow (bf16 operands, fp32 PSUM accumulation):
  - Everything stays in "transposed space" so every matmul contracts over the
    partition dim with no on-device transposes:
      x_t    [H, S]      = hidden[b].T                      (host-transposed)
      qk_T   [512, S]    = (Wqk_g x_t)                      rows: q(4 heads), k(4 heads)
      v      [S, 256]    = x w_v.T  (natural layout; lhsT = x_t chunks)
      st     [128k, q]   = k_T_h^T-contracted scores (transposed scores)
      pt     = exp(st * scale + mask[k])                    (ACT, bias = per-partition mask)
      av     [128, q]    = v_aug^T pt ; rows 0:64 = unnormalized out.T,
                           rows 64:128 = Z[q] replicated (v_aug cols 64:128 == 1)
      attn_T [256, S]    = av[:64] * reciprocal(av[64:128])
      out_t  [H, S]      = Wo_g^T-contracted partial output (transposed)
  - Host sums the 4 group partials per batch, transposes back, and adds the
    exact bias corrections: b_out plus w_out @ b_v (the ones-augmented-V
    identity makes the v-bias a constant channel offset).
"""

import numpy as np

import concourse.tile as tile
from concourse import bacc, mybir
from concourse.bass_utils import run_bass_kernel_spmd

B, S, H = 2, 2048, 1024
NH, HD = 16, 64
NCORES = 8
NGROUP = 4              # head groups = cores per batch
HPG = NH // NGROUP      # 4 heads per group
DG = HPG * HD           # 256 channels per group
P = 128
SCALE = float(HD) ** -0.5

FP32 = mybir.dt.float32
FP32R = mybir.dt.float32r
BF16 = mybir.dt.bfloat16

S_TILES = S // P        # 16 key/token tiles
WCOL = {0: 0, 2: 1, 1: 2, 3: 3}  # qk row-chunk -> wqk_t column chunk
HC = H // P             # 8 contraction chunks over H
QKR = 2 * DG            # 512 q+k rows
QKC = QKR // P          # 4 chunks of qk rows
TQ = 512                # token quarter for qkv streaming
NQT = S // TQ           # 4
QT = 1024               # q tile width in attention / out-proj
NQ = S // QT            # 2

_NC_CACHE = None
LAST_RESULT = None      # BassKernelResults of the most recent run (for test.py)


def _body(tc, x_t, wqk_t, wv_t, wo_t, bqk, mask, out_t):
    nc = tc.nc
    with (
        tc.tile_pool(name="const", bufs=1) as const,
        tc.tile_pool(name="big", bufs=1) as big,
        tc.tile_pool(name="pt_pool", bufs=24) as pt_pool,
        tc.tile_pool(name="rz_pool", bufs=2) as rz_pool,
        tc.tile_pool(name="osb_pool", bufs=3) as osb_pool,
        tc.tile_pool(name="ps", bufs=2, space="PSUM") as ps,
        tc.tile_pool(name="avps", bufs=2, space="PSUM") as avps,
        tc.tile_pool(name="iops", bufs=2, space="PSUM") as iops,
    ):
        # ---------- input DMAs ----------
        # wqk FIRST (first matmuls need it); x chunks split over sync/vector
        # so they land progressively for the hc-major prologue; nothing
        # latency-critical on gpsimd (its engine boot is ~10us).
        # byte-balanced across the three HWDGE engines; x chunks staggered
        # so the hc-major prologue consumes them in arrival order.
        wqk_sb = const.tile([P, HC, QKR], BF16, name="wqk_sb")
        wv_sb = const.tile([P, HC, DG], BF16, name="wv_sb")
        bqk_sb = const.tile([P, QKC], FP32, name="bqk_sb")
        mask_sb = const.tile([P, S_TILES], FP32, name="mask_sb")
        wo_sb = const.tile([P, DG // P, H], BF16, name="wo_sb")
        x_sb = big.tile([P, HC, S], BF16, name="x_sb")
        x_r = x_t.rearrange("(c p) s -> p c s", p=P)

        def xdma(eng, hc):
            eng.dma_start(x_sb[:, hc, :], x_r[:, hc, :])

        wqk_r = wqk_t.rearrange("(c p) r -> p c r", p=P)
        nc.sync.dma_start(wqk_sb[:], wqk_r[:])
        nc.scalar.dma_start(wv_sb[:], wv_t.rearrange("(c p) r -> p c r", p=P))
        x_eng3 = (nc.sync, nc.scalar, nc.gpsimd)
        for hc in range(HC):
            xdma(x_eng3[hc % 3], hc)
        nc.gpsimd.dma_start(bqk_sb[:], bqk.rearrange("(c p) -> p c", p=P))
        nc.gpsimd.dma_start(mask_sb[:], mask.rearrange("(c p) -> p c", p=P))
        nc.gpsimd.dma_start(wo_sb[:], wo_t.rearrange("(c p) r -> p c r", p=P))

        qk_sb = big.tile([P, QKC, S], BF16, name="qk_sb")
        # v_aug: per token tile / head: [v (64 cols) | ones (64 cols)]
        v_sb = big.tile([P, S_TILES, HPG, 2 * HD], BF16, name="v_sb")
        attn_sb = big.tile([P, DG // P, S], BF16, name="attn_sb")

        # ones half of v_aug: memset a bf16 staging tile, copy per token tile
        ones_sb = const.tile([P, HPG, HD], BF16, name="ones_sb")
        nc.vector.memset(ones_sb[:], 1.0)
        for tt in range(S_TILES):
            nc.vector.tensor_copy(v_sb[:, tt, :, HD:2 * HD], ones_sb[:])

        # ---------- hc-major prologue ----------
        # Only what the very first score tiles need: q pair0 window 0
        # (qk rc=0 i=0) and k pair0 tiles 0-7 (qk rc=2 i=0,1), accumulated
        # chunk-major so each x chunk is consumed as its DMA lands.  The
        # rest of the projections stream in as metered filler below.
        pro_a = ps.tile([P, QT], FP32, name="st", tag="mm")
        pro_b = ps.tile([P, QT], FP32, name="st", tag="mm")
        for hc in range(HC):
            se = (hc == 0, hc == HC - 1)
            nc.tensor.matmul(pro_a[:, 0:512], lhsT=wqk_sb[:, hc, 0:P],
                             rhs=x_sb[:, hc, 0:512], start=se[0], stop=se[1])
            nc.tensor.matmul(pro_a[:, 512:1024],
                             lhsT=wqk_sb[:, hc, P:2 * P],
                             rhs=x_sb[:, hc, 0:512], start=se[0], stop=se[1])
            nc.tensor.matmul(pro_b[:, 0:512],
                             lhsT=wqk_sb[:, hc, P:2 * P],
                             rhs=x_sb[:, hc, 512:1024], start=se[0],
                             stop=se[1])
        nc.vector.tensor_scalar_add(qk_sb[:, 0, 0:512], pro_a[:, 0:512],
                                    bqk_sb[:, 0:1])
        nc.vector.tensor_scalar_add(qk_sb[:, 2, 0:512], pro_a[:, 512:1024],
                                    bqk_sb[:, 2:3])
        nc.vector.tensor_scalar_add(qk_sb[:, 2, 512:1024], pro_b[:, 0:512],
                                    bqk_sb[:, 2:3])

        # ---------- static micro-scheduler ----------
        # One flat PE instruction stream: score(t) every ~997ns (the exp
        # cadence); everything else (remaining qkv projection, AV
        # accumulation, out-proj) is sliced into <=2-matmul pieces and
        # packed into the gaps under an explicit PE-time budget, so the
        # in-order PE FIFO never delays a score (and hence never starves
        # the ACT exp stream).
        MM512 = 0.215   # us, N=512 matmul issue slot (warm)
        MM256 = 0.110
        EXPP = 0.997    # exp cadence
        o_r = out_t.rearrange("(c p) s -> p c s", p=P)
        NT = 8 * S_TILES

        def win(w):
            return (0, w) if w < 4 else (1, w - 4)

        # --- filler piece generators (qk / v groups sliced hc-pair-wise) ---
        filler_q = []   # (cost_us, fn)

        def gen_qk(rc, i):
            st_ = {}

            def mk(h0):
                def f():
                    if "t" not in st_:
                        st_["t"] = iops.tile([P, 512], FP32, name="qk_ps",
                                             tag="io")
                    for hc in (h0, h0 + 1):
                        nc.tensor.matmul(
                            st_["t"][:],
                            lhsT=wqk_sb[:, hc,
                                        WCOL[rc] * P:(WCOL[rc] + 1) * P],
                            rhs=x_sb[:, hc, i * 512:(i + 1) * 512],
                            start=(hc == 0), stop=(hc == HC - 1))
                return f
            for h0 in range(0, HC, 2):
                filler_q.append((2 * MM512, mk(h0)))

            def fin():
                nc.vector.tensor_scalar_add(
                    qk_sb[:, rc, i * 512:(i + 1) * 512], st_["t"][:],
                    bqk_sb[:, rc:rc + 1])
                done_groups.add(("qk", rc, i))
            filler_q.append((0.0, fin))

        v_ready = {}    # token tile -> modeled pe time its SBUF copy lands
        done_groups = {("qk", 0, 0), ("qk", 2, 0), ("qk", 2, 1)}

        def gen_v(tp):
            st_ = {}

            def mk(half, h0):
                def f():
                    if "t" not in st_:
                        st_["t"] = iops.tile([P, 512], FP32, name="v_ps",
                                             tag="io")
                    tt = 2 * tp + half
                    for hc in (h0, h0 + 1):
                        nc.tensor.matmul(
                            st_["t"][:, half * DG:(half + 1) * DG],
                            lhsT=x_sb[:, hc, tt * P:(tt + 1) * P],
                            rhs=wv_sb[:, hc, :],
                            start=(hc == 0), stop=(hc == HC - 1))
                return f
            for half in range(2):
                for h0 in range(0, HC, 2):
                    filler_q.append((2 * MM256, mk(half, h0)))

            def fin():
                nc.vector.tensor_copy(
                    v_sb[:, 2 * tp:2 * tp + 2, :, 0:HD],
                    st_["t"][:].rearrange("p (t h d) -> p t h d", t=2, d=HD))
                v_ready[2 * tp] = v_ready[2 * tp + 1] = pe[0] + 0.9
            filler_q.append((0.0, fin))

        # filler order: k tiles 8-15 first (scores slots 8-15 need them),
        # v in token order (AV consumption), q chunks before their windows,
        # pair-1 k before slot 64.
        gen_qk(2, 2)
        gen_qk(2, 3)
        for tp in (0, 1, 2):
            gen_v(tp)
        gen_qk(0, 1)
        for tp in (3, 4):
            gen_v(tp)
        gen_qk(0, 2)
        for tp in (5, 6, 7):
            gen_v(tp)
        gen_qk(0, 3)
        for j in range(4):
            gen_qk(3, j)
        for j in range(4):
            gen_qk(1, j)

        # --- scheduler state ---
        pe = [0.0]          # modeled PE-busy time since stream start
        E = [-EXPP]         # E[t+1] = modeled end of exp(t)
        cur_t = [0]         # current slot (wall-clock proxy for gates)
        avs = {}
        pts = {}
        av_q = []           # pending AV units: (t, kt, w)
        op_q = []           # pending out-proj: (ready_pe, ready_slot, q5, j)
        fin_pe = {}
        fin_slot = {}

        def emit_score_exp(t):
            w, kt = divmod(t, S_TILES)
            qc, q5 = win(w)
            qlo = q5 * 512
            st = ps.tile([P, QT], FP32, name="st", tag="mm")
            for half in range(2):
                off = half * HD
                nc.tensor.matmul(
                    st[:, half * 512:(half + 1) * 512],
                    lhsT=qk_sb[off:off + HD, 2 + qc, kt * P:(kt + 1) * P],
                    rhs=qk_sb[off:off + HD, qc, qlo:qlo + 512],
                    start=True, stop=True)
            pe[0] += MM512
            pt = pt_pool.tile([P, QT], BF16, name="pt", tag="pt")
            nc.scalar.activation(
                pt[:], st[:], mybir.ActivationFunctionType.Exp,
                bias=mask_sb[:, kt:kt + 1], scale=SCALE)
            pts[t] = pt
            E.append(max(E[-1] + EXPP, pe[0] + 0.45))

        def av_ready(unit):
            t, kt, w = unit
            if pe[0] + 0.1 < E[t + 1]:          # exp(t) must have completed
                return False
            if v_ready.get(kt, 1e9) > pe[0]:    # v tile must be in SBUF
                return False
            if kt == 0 and w > 0:               # av slots: window w-1's
                f = fin_pe.get(w - 1)           # DVE chain must have run
                if f is None or pe[0] < f + 3.4:
                    return False
            return True

        def emit_av(unit):
            t, kt, w = unit
            qc, q5 = win(w)
            if kt == 0:
                avs[w] = (avps.tile([P, 512], FP32, name="av0", tag="av"),
                          avps.tile([P, 512], FP32, name="av1", tag="av"))
            pt = pts.pop(t)
            for half, av in ((0, avs[w][0]), (1, avs[w][1])):
                nc.tensor.matmul(
                    av[:], lhsT=v_sb[:, kt, 2 * qc + half, :],
                    rhs=pt[:, half * 512:(half + 1) * 512],
                    start=(kt == 0), stop=(kt == S_TILES - 1))
            pe[0] += 2 * MM512
            if kt == S_TILES - 1:
                emit_finalize(w, tail=(w == 7))

        def emit_finalize(w, tail=False):
            qc, q5 = win(w)
            qlo = q5 * 512
            for half, av in ((0, avs[w][0]), (1, avs[w][1])):
                off = half * HD
                zc = rz_pool.tile([HD, 512], FP32, name="zc", tag="zc")
                if tail:
                    nc.scalar.copy(zc[:], av[HD:2 * HD, :])
                else:
                    nc.vector.tensor_copy(zc[:], av[HD:2 * HD, :])
                rz = rz_pool.tile([HD, 512], FP32, name="rz", tag="rz")
                nc.vector.reciprocal_approx_fast(rz[:], zc[:])
                nc.vector.tensor_mul(
                    attn_sb[off:off + HD, qc, qlo:qlo + 512],
                    av[0:HD, :], rz[:])
            del avs[w]
            fin_pe[w] = pe[0]
            fin_slot[w] = cur_t[0]
            if w >= 4:
                q5o = w - 4
                for j in range(H // P):
                    op_q.append((pe[0] + 3.0, cur_t[0] + 4, q5o, j))

        def op_ready(unit):
            return pe[0] >= unit[0]

        def emit_op(unit, tail=False):
            _, _, q5, j = unit
            qlo = q5 * 512
            o_ps = iops.tile([P, 512], FP32, name="o_ps", tag="io")
            for kc in range(DG // P):
                nc.tensor.matmul(
                    o_ps[:], lhsT=wo_sb[:, kc, j * P:(j + 1) * P],
                    rhs=attn_sb[:, kc, qlo:qlo + 512],
                    start=(kc == 0), stop=(kc == DG // P - 1))
            pe[0] += 2 * MM512
            o_sb = osb_pool.tile([P, 512], BF16, name="o_sb", tag="osb")
            if tail:
                nc.scalar.copy(o_sb[:], o_ps[:])
            else:
                nc.vector.tensor_copy(o_sb[:], o_ps[:])
            nc.sync.dma_start(o_r[:, j, qlo:qlo + 512], o_sb[:])

        def fill_until(deadline):
            # round-robin under the exp-cadence budget: AV first (pt-slot
            # recycling gates the exp stream), filler second, out-proj when
            # its window is done.  Then a small unconditional filler quota
            # so projection prerequisites never pile into a forced burst.
            while True:
                did = False
                if av_q and av_ready(av_q[0]):
                    if pe[0] + 2 * MM512 <= deadline:
                        emit_av(av_q.pop(0)); did = True
                if not did and filler_q:
                    if pe[0] + filler_q[0][0] <= deadline:
                        cost, fn = filler_q.pop(0)
                        fn(); pe[0] += cost
                        did = True
                if not did and op_q and op_ready(op_q[0]):
                    if pe[0] + 2 * MM512 <= deadline:
                        emit_op(op_q.pop(0)); did = True
                if not did:
                    break

        def need_group(key):
            # force-drain filler (in order) until `key`'s finishing piece
            # has been emitted — scores must never precede their qk writes
            # in emission order (Tile deps follow emission order).
            while key not in done_groups:
                cost, fn = filler_q.pop(0)
                fn(); pe[0] += cost

        for t in range(NT):
            cur_t[0] = t
            w, kt = divmod(t, S_TILES)
            qc, q5 = win(w)
            need_group(("qk", qc, q5))
            need_group(("qk", 2 + qc, kt // 4))
            emit_score_exp(t)
            av_q.append((t, kt, w))
            fill_until(E[-1] - 0.32)

        # drain remaining AV / out-proj work
        while av_q:
            cur_t[0] += 1
            if av_ready(av_q[0]):
                emit_av(av_q.pop(0))
            elif filler_q:
                cost, fn = filler_q.pop(0)
                fn(); pe[0] += cost
            else:
                pe[0] = max(pe[0] + 0.2, E[av_q[0][0] + 1])
        while filler_q:
            cost, fn = filler_q.pop(0)
            fn(); pe[0] += cost
        for unit in op_q:
            emit_op(unit, tail=(unit[2] == 3))
        op_q.clear()


def _build():
    nc = bacc.Bacc(
        "TRN2",
        target_bir_lowering=False,
        debug=False,
        enable_asserts=True,
        num_devices=NCORES,
    )
    x_t = nc.dram_tensor("x_t", [H, S], BF16, kind="ExternalInput").ap()
    wqk_t = nc.dram_tensor("wqk_t", [H, QKR], BF16, kind="ExternalInput").ap()
    wv_t = nc.dram_tensor("wv_t", [H, DG], BF16, kind="ExternalInput").ap()
    wo_t = nc.dram_tensor("wo_t", [DG, H], BF16, kind="ExternalInput").ap()
    bqk = nc.dram_tensor("bqk", [QKR], FP32, kind="ExternalInput").ap()
    mask = nc.dram_tensor("mask", [S], FP32, kind="ExternalInput").ap()
    out_t = nc.dram_tensor("out_t", [H, S], BF16, kind="ExternalOutput").ap()

    with tile.TileContext(nc) as tc:
        _body(tc, x_t, wqk_t, wv_t, wo_t, bqk, mask, out_t)
    nc.compile()
    return nc


def _get_nc():
    global _NC_CACHE
    if _NC_CACHE is None:
        _NC_CACHE = _build()
    return _NC_CACHE


def make_in_maps(hidden_states, attention_mask, w_qkv, b_qkv, w_out):
    import ml_dtypes

    bf16 = ml_dtypes.bfloat16
    in_maps = []
    for core in range(NCORES):
        b, g = divmod(core, NGROUP)
        wq = w_qkv[0 * H + g * DG:0 * H + (g + 1) * DG]
        wk = w_qkv[1 * H + g * DG:1 * H + (g + 1) * DG]
        wv = w_qkv[2 * H + g * DG:2 * H + (g + 1) * DG]
        in_maps.append({
            "x_t": np.ascontiguousarray(hidden_states[b].T).astype(bf16),
            # column order [q0|k0|q1|k1] (128-row chunks interleaved) so
            # the kernel can DMA the first-needed half contiguously
            "wqk_t": np.ascontiguousarray(
                np.concatenate([wq[:128], wk[:128], wq[128:], wk[128:]],
                               0).T).astype(bf16),
            "wv_t": np.ascontiguousarray(wv.T).astype(bf16),
            "wo_t": np.ascontiguousarray(
                w_out[:, g * DG:(g + 1) * DG].T).astype(bf16),
            "bqk": np.ascontiguousarray(
                np.concatenate([b_qkv[g * DG:(g + 1) * DG],
                                b_qkv[H + g * DG:H + (g + 1) * DG]])),
            "mask": np.ascontiguousarray(attention_mask[b]),
        })
    return in_maps


def kernel(hidden_states, attention_mask, w_qkv, b_qkv, w_out, b_out):
    global LAST_RESULT
    hidden_states = np.asarray(hidden_states, dtype=np.float32)
    attention_mask = np.asarray(attention_mask, dtype=np.float32)
    w_qkv = np.asarray(w_qkv, dtype=np.float32)
    b_qkv = np.asarray(b_qkv, dtype=np.float32)
    w_out = np.asarray(w_out, dtype=np.float32)
    b_out = np.asarray(b_out, dtype=np.float32)

    nc = _get_nc()
    in_maps = make_in_maps(hidden_states, attention_mask, w_qkv, b_qkv, w_out)

    import os
    trace = bool(int(os.environ.get("KERNEL_TRACE", "0")))
    res = run_bass_kernel_spmd(
        nc, in_maps, core_ids=list(range(NCORES)), trace=trace,
    )
    LAST_RESULT = res

    out = np.zeros((B, S, H), np.float32)
    vbias = w_out @ b_qkv[2 * H:]          # exact v-bias correction
    for b in range(B):
        acc = res.results[b * NGROUP + 0]["out_t"].astype(np.float32)
        for g in range(1, NGROUP):
            acc = acc + res.results[b * NGROUP + g]["out_t"].astype(np.float32)
        out[b] = acc.T + b_out + vbias
    return out

